# revision 12
# baseline (speedup 1.0000x reference)
"""Trainium2 Bass kernel for nn_Net_15487652069990.

700-step recurrent NEF-oscillator + limb simulation, data-parallel over 8
NeuronCores (512 batch elements per core).

Layout: features on partitions, batch on the free dim (N=512) for the big
matmuls; limb state is kept batch-major (128 partitions x 4 chunks) so the
elementwise limb integration runs as wide DVE ops.

Per step (per core):
  - 16 block-diag encoder matmuls (K=101 incl. ones-row bias fold) -> PSUM
  - relu PSUM->SBUF bf16 (alternating ScalarE/VectorE)
  - 16 decoder matmuls (DT prefolded) accumulating deriv in PSUM
  - 16 fc4 matmuls accumulating in PSUM
  - os += deriv (VectorE, fp32)
  - b1 = relu(fc4+b4) (ScalarE), 4 batch-major fc5 matmuls, + direct
  - limb integration: 13 small VectorE ops on (128, 4/8) tiles
Outputs are accumulated in SBUF history tiles and flushed to DRAM every
H steps.
"""

import sys

if "/opt/trn_rl_repo" not in sys.path:
    sys.path.insert(0, "/opt/trn_rl_repo")

import numpy as np
import ml_dtypes

# ---- problem constants (hardcoded from the problem spec) ----
N_OSC = 50
N_PER = 40
DT = 0.001
STEPS = 700
HALF_PI = float(np.pi / 2)
R_MOM = 0.02
L0 = 0.1
INERTIA = 1e-3
DAMP = 0.01
K_MUS = 50.0
L_REST = 0.05
TAU_M = 0.05

B = 4096
NCORES = 8
BC = B // NCORES          # 512 batch per core
F = N_OSC * N_PER         # 2000 flat (osc, per) index
NCH = 16                  # chunks over F
CH = F // NCH             # 125 real rows per chunk
BCH = 4                   # batch chunks of 128 (batch-major ops)

# derived limb constants
C_OM = 1.0 - DT * DAMP / INERTIA            # 0.99
C_F = K_MUS * DT * R_MOM / INERTIA          # 1.0 (folds K_MUS into force diff)
C_L = 1.0 - DT / TAU_M                      # 0.98
C_LT = DT * R_MOM / TAU_M                   # 4e-4
C_LB = DT * L0 / TAU_M                      # 2e-3

F32 = np.float32
BF16 = ml_dtypes.bfloat16

# dtype switches (perf/precision knobs)
ENC_DTYPE = "float32"     # encoder matmul dtype (os + enc weights)
A_DTYPE = "bfloat16"      # activities dtype in SBUF (dec/fc4 rhs)
W_DTYPE = "bfloat16"      # dec/fc4/fc5 weight dtype


def _prep_weights(inputs):
    """Host-side numpy preprocessing of all weight tensors into the DRAM
    layouts the device kernel consumes. All 2D (partition, free)."""
    enc = np.asarray(inputs["enc"], F32)            # (O, P, 2)
    osc_bias = np.asarray(inputs["osc_bias"], F32)  # (O, P)
    dec = np.asarray(inputs["dec"], F32)            # (O, 2, P)
    fc1_w = np.asarray(inputs["fc1_w"], F32)        # (128, 2)
    fc1_b = np.asarray(inputs["fc1_b"], F32)
    fc2_w = np.asarray(inputs["fc2_w"], F32)        # (128, 128)
    fc2_b = np.asarray(inputs["fc2_b"], F32)
    fc3_w = np.asarray(inputs["fc3_w"], F32)        # (100, 128)
    fc3_b = np.asarray(inputs["fc3_b"], F32)
    fcd_w = np.asarray(inputs["fcd_w"], F32)        # (2, 128)
    fcd_b = np.asarray(inputs["fcd_b"], F32)
    fc4_w = np.asarray(inputs["fc4_w"], F32)        # (32, 2000)
    fc4_b = np.asarray(inputs["fc4_b"], F32)
    fc5_w = np.asarray(inputs["fc5_w"], F32)        # (2, 32)
    fc5_b = np.asarray(inputs["fc5_b"], F32)

    enc_np = {"float32": F32, "bfloat16": BF16}[ENC_DTYPE]
    w_np = {"float32": F32, "bfloat16": BF16}[W_DTYPE]

    # encoder lhsT: (101, NCH*128). col c*128+m -> a_pre row j=125c+m.
    # row k=2o+d -> os row; row 100 = osc_bias (ones row of os).
    encT = np.zeros((101, NCH * 128), F32)
    j = np.arange(F)
    o = j // N_PER
    p = j % N_PER
    cc = j // CH
    m = j % CH
    col = cc * 128 + m
    encT[2 * o, col] = enc[o, p, 0]
    encT[2 * o + 1, col] = enc[o, p, 1]
    encT[100, col] = osc_bias[o, p]

    # decoder lhsT (DT folded): (128, NCH*100). row r (a row), col c*100 + (2o+d)
    decT = np.zeros((128, NCH * 100), F32)
    decT[m, cc * 100 + 2 * o] = DT * dec[o, 0, p]
    decT[m, cc * 100 + 2 * o + 1] = DT * dec[o, 1, p]

    # fc4 lhsT: (128, NCH*32)
    w4T = np.zeros((128, NCH * 32), F32)
    for c in range(NCH):
        w4T[:CH, c * 32:(c + 1) * 32] = fc4_w[:, c * CH:(c + 1) * CH].T

    w5T = fc5_w.T.copy()                             # (32, 2)

    w1T_aug = np.concatenate([fc1_w.T, fc1_b[None, :]], 0)  # (3, 128)
    w2T = fc2_w.T.copy()                             # (128, 128)
    b2c = fc2_b[:, None].copy()                      # (128, 1)
    w3T = fc3_w.T.copy()                             # (128, 100)
    b3c = fc3_b[:, None].copy()                      # (100, 1)
    wdT = fcd_w.T.copy()                             # (128, 2)
    b4c = fc4_b[:, None].copy()                      # (32, 1)
    # direct bias broadcast, batch-major (128, 8): col 2c+k -> bias k
    dirb = np.zeros((128, 8), F32)
    dirb[:, 0::2] = fcd_b[0] + fc5_b[0]
    dirb[:, 1::2] = fcd_b[1] + fc5_b[1]

    return {
        "encT": encT.astype(enc_np),
        "decT": decT.astype(w_np),
        "w4T": w4T.astype(w_np),
        "w5T": w5T.astype(w_np),
        "w1T_aug": w1T_aug,
        "w2T": w2T,
        "b2c": b2c,
        "w3T": w3T,
        "b3c": b3c,
        "wdT": wdT,
        "b4c": b4c,
        "dirb": dirb,
    }


def _prep_x(x_shard):
    """Per-core input data layouts."""
    x_shard = np.asarray(x_shard, F32)               # (512, 2)
    xT_aug = np.empty((3, BC), F32)
    xT_aug[0] = x_shard[:, 0]
    xT_aug[1] = x_shard[:, 1]
    xT_aug[2] = 1.0
    theta0 = x_shard[:, 0].reshape(BCH, 128).T.copy()  # (128, 4)
    return {"xT_aug": xT_aug, "theta0": theta0}


def _install_wait_split_patch():
    """This container's walrus build only supports ONE semaphore wait per
    engine instruction ("Too many sync wait commands" in setupSyncWait).
    Tile emits instructions with several waits. Rewrite the BIR before
    walrus: hoist all-but-one waits of each instruction into single-wait
    NoOps on the same engine immediately before it (engines execute their
    streams in order, so this is semantically identical)."""
    import json as _json
    import concourse.bass_utils as bu
    import concourse.bass2jax as b2j

    if getattr(bu, "_wait_split_installed", False):
        return
    orig = bu.compile_bir_kernel

    def split_waits_in_bir(bir_str):
        was_bytes = isinstance(bir_str, (bytes, bytearray))
        d = _json.loads(bir_str)
        ctr = 0
        for fn in d.get("functions", []):
            for blk in fn.get("blocks", []):
                new_insts = []
                for inst in blk.get("instructions", []):
                    si = inst.get("sync_info") or {}
                    waits = si.get("on_wait") or []
                    if len(waits) > 1:
                        for w in waits[:-1]:
                            ctr += 1
                            nop = {
                                "engine": inst["engine"],
                                "ins": [],
                                "outs": [],
                                "name": f"{inst['name']}-ws{ctr}",
                                "opcode": "NoOp",
                                "sync_info": {"on_update": [], "on_wait": [w]},
                            }
                            if "debug" in inst:
                                nop["debug"] = inst["debug"]
                            new_insts.append(nop)
                        si = dict(si)
                        si["on_wait"] = [waits[-1]]
                        inst = dict(inst)
                        inst["sync_info"] = si
                    new_insts.append(inst)
                blk["instructions"] = new_insts
        out = _json.dumps(d)
        return out.encode() if was_bytes else out

    def patched(bir_str, *a, **kw):
        return orig(split_waits_in_bir(bir_str), *a, **kw)

    bu.compile_bir_kernel = patched
    b2j.compile_bir_kernel = patched
    bu._wait_split_installed = True


def _build_program(steps=STEPS, hist=50):
    import concourse.bass as bass
    import concourse.mybir as mybir
    from concourse.tile import TileContext

    dt = mybir.dt
    enc_dt = getattr(dt, ENC_DTYPE)
    a_dt = getattr(dt, A_DTYPE)
    w_dt = getattr(dt, W_DTYPE)
    AF = mybir.ActivationFunctionType
    OP = mybir.AluOpType

    nc = bass.Bass()

    # ---- DRAM I/O ----
    d_encT = nc.dram_tensor("encT", (101, NCH * 128), enc_dt, kind="ExternalInput")
    d_decT = nc.dram_tensor("decT", (128, NCH * 100), w_dt, kind="ExternalInput")
    d_w4T = nc.dram_tensor("w4T", (128, NCH * 32), w_dt, kind="ExternalInput")
    d_w5T = nc.dram_tensor("w5T", (32, 2), w_dt, kind="ExternalInput")
    d_w1T = nc.dram_tensor("w1T_aug", (3, 128), dt.float32, kind="ExternalInput")
    d_w2T = nc.dram_tensor("w2T", (128, 128), dt.float32, kind="ExternalInput")
    d_b2c = nc.dram_tensor("b2c", (128, 1), dt.float32, kind="ExternalInput")
    d_w3T = nc.dram_tensor("w3T", (128, 100), dt.float32, kind="ExternalInput")
    d_b3c = nc.dram_tensor("b3c", (100, 1), dt.float32, kind="ExternalInput")
    d_wdT = nc.dram_tensor("wdT", (128, 2), dt.float32, kind="ExternalInput")
    d_b4c = nc.dram_tensor("b4c", (32, 1), dt.float32, kind="ExternalInput")
    d_dirb = nc.dram_tensor("dirb", (128, 8), dt.float32, kind="ExternalInput")
    d_xT = nc.dram_tensor("xT_aug", (3, BC), dt.float32, kind="ExternalInput")
    d_th0 = nc.dram_tensor("theta0", (128, BCH), dt.float32, kind="ExternalInput")

    # outputs mirror SBUF (partition, free) layout; host reorders.
    d_lout = nc.dram_tensor("l_out", (128, steps * 16), dt.float32,
                            kind="ExternalOutput")
    d_aout = nc.dram_tensor("a_out", (128, steps * 8), dt.float32,
                            kind="ExternalOutput")

    with TileContext(nc) as tc:
        with (
            tc.tile_pool(name="consts", bufs=1) as cpool,
            tc.tile_pool(name="apool", bufs=2) as apool,
            tc.tile_pool(name="b1pool", bufs=2) as b1pool,
            tc.tile_pool(name="tmp", bufs=10) as tpool,
            tc.tile_pool(name="lh", bufs=2) as lhpool,
            tc.tile_pool(name="ah", bufs=2) as ahpool,
            tc.tile_pool(name="pa", bufs=3, space="PSUM") as papool,
            tc.tile_pool(name="pd", bufs=2, space="PSUM") as pdpool,
            tc.tile_pool(name="pb", bufs=2, space="PSUM") as pbpool,
            tc.tile_pool(name="pact", bufs=1, space="PSUM") as pactpool,
        ):
            # ---- load constants ----
            def load(dram, shape, dtype, tag):
                t = cpool.tile(list(shape), dtype, tag=tag)
                nc.sync.dma_start(t[:, :], dram[:, :])
                return t

            encT = load(d_encT, (101, NCH * 128), enc_dt, "encT")
            decT = load(d_decT, (128, NCH * 100), w_dt, "decT")
            w4T = load(d_w4T, (128, NCH * 32), w_dt, "w4T")
            w5T = load(d_w5T, (32, 2), w_dt, "w5T")
            w1T = load(d_w1T, (3, 128), dt.float32, "w1T")
            w2T = load(d_w2T, (128, 128), dt.float32, "w2T")
            b2c = load(d_b2c, (128, 1), dt.float32, "b2c")
            w3T = load(d_w3T, (128, 100), dt.float32, "w3T")
            b3c = load(d_b3c, (100, 1), dt.float32, "b3c")
            wdT = load(d_wdT, (128, 2), dt.float32, "wdT")
            b4c = load(d_b4c, (32, 1), dt.float32, "b4c")
            dirb = load(d_dirb, (128, 8), dt.float32, "dirb")
            xT = load(d_xT, (3, BC), dt.float32, "xT")

            os_sb = cpool.tile([101, BC], enc_dt, tag="os")
            direct = cpool.tile([128, 8], dt.float32, tag="direct")
            linit = cpool.tile([128, 16], dt.float32, tag="linit")

            # ---- preamble ----
            ph = papool.tile([128, BC], dt.float32, tag="pa")
            nc.tensor.matmul(ph[:, :], w1T[:, :], xT[:, :], start=True, stop=True)
            h_sb = tpool.tile([128, BC], dt.float32, tag="h")
            nc.scalar.activation(h_sb[:, :], ph[:, :], AF.Relu)

            # direct (batch-major): 4 chunks of fcd + bias
            pdir = pactpool.tile([128, 8], dt.float32, tag="pact")
            for c in range(BCH):
                nc.tensor.matmul(pdir[:, 2 * c:2 * c + 2],
                                 h_sb[:, c * 128:(c + 1) * 128],
                                 wdT[:, :], start=True, stop=True)
            nc.vector.tensor_add(direct[:, :], pdir[:, :], dirb[:, :])

            ph2 = papool.tile([128, BC], dt.float32, tag="pa")
            nc.tensor.matmul(ph2[:, :], w2T[:, :], h_sb[:, :], start=True, stop=True)
            h2_sb = tpool.tile([128, BC], dt.float32, tag="h2")
            nc.scalar.activation(h2_sb[:, :], ph2[:, :], AF.Relu, bias=b2c[:, :])

            pos = pdpool.tile([100, BC], dt.float32, tag="pd")
            nc.tensor.matmul(pos[:, :], w3T[:, :], h2_sb[:, :], start=True, stop=True)
            # ones row first (partition base must be 0/32/64/96): fill 96:101,
            # then the real os rows 0:100 overwrite 96:100.
            nc.vector.memset(os_sb[96:101, :], 1.0)
            nc.scalar.add(os_sb[0:100, :], pos[:, :], b3c[:, :])

            # limb init: theta, omega=0, l1, l2
            nc.sync.dma_start(linit[:, 0:BCH], d_th0[:, :])
            nc.vector.memset(linit[:, 4:8], 0.0)
            nc.vector.tensor_scalar(linit[:, 8:12], linit[:, 0:4],
                                    -R_MOM, L0, op0=OP.mult, op1=OP.add)
            nc.vector.tensor_scalar(linit[:, 12:16], linit[:, 0:4],
                                    R_MOM, L0, op0=OP.mult, op1=OP.add)

            # barrier so step-0 instructions don't stack waits on all the
            # preamble DMA queues + engines (HW cap on per-inst sync waits)
            tc.strict_bb_all_engine_barrier()

            # ---- main loop ----
            prev_l = linit
            lh_tile = None
            ah_tile = None
            n_flush = steps // hist
            assert steps % hist == 0

            for t in range(steps):
                s = t % hist
                if s == 0:
                    lh_tile = lhpool.tile([128, 16 * hist], dt.float32, tag="lh")
                    ah_tile = ahpool.tile([128, 8 * hist], dt.float32, tag="ah")

                a_all = apool.tile([128, NCH * BC], a_dt, tag="a")

                # encoder matmuls + relu
                pas = []
                for c in range(NCH):
                    pa = papool.tile([128, BC], dt.float32, tag="pa")
                    nc.tensor.matmul(pa[:, :], encT[:, c * 128:(c + 1) * 128],
                                     os_sb[:, :], start=True, stop=True)
                    pas.append(pa)
                    a_sl = a_all[:, c * BC:(c + 1) * BC]
                    if c % 2 == 0:
                        nc.scalar.activation(a_sl, pa[:, :], AF.Relu)
                    else:
                        nc.vector.tensor_scalar_max(a_sl, pa[:, :], 0.0)

                # decoder + fc4 accumulation
                pd = pdpool.tile([100, BC], dt.float32, tag="pd")
                pb = pbpool.tile([32, BC], dt.float32, tag="pb")
                for c in range(NCH):
                    a_sl = a_all[:, c * BC:(c + 1) * BC]
                    nc.tensor.matmul(pd[:, :], decT[:, c * 100:(c + 1) * 100],
                                     a_sl, start=(c == 0), stop=(c == NCH - 1))
                    nc.tensor.matmul(pb[:, :], w4T[:, c * 32:(c + 1) * 32],
                                     a_sl, start=(c == 0), stop=(c == NCH - 1))

                # os += DT * deriv  (deriv prefolded with DT)
                nc.vector.tensor_add(os_sb[0:100, :], os_sb[0:100, :], pd[:, :])

                # readout: b1 = relu(fc4 + b4); act = b1 @ w5T + direct (batch-major)
                b1 = b1pool.tile([32, BC], a_dt, tag="b1")
                nc.scalar.activation(b1[:, :], pb[:, :], AF.Relu, bias=b4c[:, :])
                pact = pactpool.tile([128, 8], dt.float32, tag="pact")
                for c in range(BCH):
                    nc.tensor.matmul(pact[:, 2 * c:2 * c + 2],
                                     b1[:, c * 128:(c + 1) * 128],
                                     w5T[:, :], start=True, stop=True)
                act = ah_tile[:, s * 8:(s + 1) * 8]
                nc.vector.tensor_add(act, pact[:, :], direct[:, :])

                # ---- limb integration (batch-major, fp32) ----
                lh = lh_tile[:, s * 16:(s + 1) * 16]
                th_p = prev_l[:, 0:4]
                om_p = prev_l[:, 4:8]
                l1_p = prev_l[:, 8:12]
                l2_p = prev_l[:, 12:16]
                l12_p = prev_l[:, 8:16]

                # act is c-major (col = 2c+k); l12 is k-major (col = k*4+c).
                # Build g12 in c-major via a rearranged read of l12.
                relu_a = tpool.tile([128, 8], dt.float32, tag="relu_a")
                nc.vector.tensor_scalar_max(relu_a[:, :], act, 0.0)
                g12 = tpool.tile([128, 8], dt.float32, tag="g12")
                l12_ck = l12_p.rearrange("p (k c) -> p c k", k=2)
                g12_ck = g12[:, :].rearrange("p (c k) -> p c k", k=2)
                nc.vector.tensor_scalar(g12_ck, l12_ck, -L_REST, 0.0,
                                        op0=OP.add, op1=OP.max)
                f12 = tpool.tile([128, 8], dt.float32, tag="f12")
                nc.vector.tensor_mul(f12[:, :], relu_a[:, :], g12[:, :])
                df = tpool.tile([128, 4], dt.float32, tag="df")
                f12_kc = f12[:, :].rearrange("p (c k) -> p k c", k=2)
                nc.vector.tensor_sub(df[:, :], f12_kc[:, 1, :], f12_kc[:, 0, :])

                th_raw = tpool.tile([128, 4], dt.float32, tag="th_raw")
                nc.vector.scalar_tensor_tensor(th_raw[:, :], om_p, DT, th_p,
                                               op0=OP.mult, op1=OP.add)
                # omega' = C_OM * omega + C_F * (f2 - f1), C_F == 1.0
                nc.vector.scalar_tensor_tensor(lh[:, 4:8], om_p, C_OM, df[:, :],
                                               op0=OP.mult, op1=OP.add)
                t1 = tpool.tile([128, 4], dt.float32, tag="t1")
                nc.vector.tensor_scalar(t1[:, :], th_p, -C_LT, C_LB,
                                        op0=OP.mult, op1=OP.add)
                nc.vector.scalar_tensor_tensor(lh[:, 8:12], l1_p, C_L, t1[:, :],
                                               op0=OP.mult, op1=OP.add)
                t2 = tpool.tile([128, 4], dt.float32, tag="t2")
                nc.vector.tensor_scalar(t2[:, :], th_p, C_LT, C_LB,
                                        op0=OP.mult, op1=OP.add)
                nc.vector.scalar_tensor_tensor(lh[:, 12:16], l2_p, C_L, t2[:, :],
                                               op0=OP.mult, op1=OP.add)
                # clip theta; zero omega unless strictly in bounds
                # (|theta_raw| < HALF_PI, strict — the boundary is absorbing)
                nc.vector.tensor_scalar(lh[:, 0:4], th_raw[:, :],
                                        HALF_PI, -HALF_PI, op0=OP.min, op1=OP.max)
                inb = tpool.tile([128, 4], dt.float32, tag="inb")
                nc.vector.tensor_scalar(inb[:, :], th_raw[:, :], HALF_PI, None,
                                        op0=OP.is_lt)
                nc.vector.scalar_tensor_tensor(inb[:, :], th_raw[:, :], -HALF_PI,
                                               inb[:, :], op0=OP.is_gt, op1=OP.mult)
                nc.vector.tensor_mul(lh[:, 4:8], lh[:, 4:8], inb[:, :])

                prev_l = lh

                # flush history
                if s == hist - 1:
                    t0 = t - hist + 1
                    nc.sync.dma_start(
                        d_lout[:, t0 * 16:(t0 + hist) * 16], lh_tile[:, :])
                    nc.sync.dma_start(
                        d_aout[:, t0 * 8:(t0 + hist) * 8], ah_tile[:, :])

    return nc


def _install_ntff_shim():
    """The agent image's antenv lacks axon_hooks; provide it using the
    boot module's ctypes NTFF driver so trace=True works."""
    import types

    try:
        import antenv.axon_hooks  # noqa
        return
    except ImportError:
        pass
    import antenv
    from trn_agent_boot.trn_boot import _ntff_profile_via_ctypes

    hook = _ntff_profile_via_ctypes("/opt/axon/libaxon_pjrt.so")
    mod = types.ModuleType("antenv.axon_hooks")
    mod.get_axon_ntff_profile_hook = lambda: hook
    mod.set_axon_ntff_profile_hook = lambda h: None
    sys.modules["antenv.axon_hooks"] = mod
    antenv.axon_hooks = mod


def _run(inputs, steps=STEPS, trace=False, hist=50):
    _install_wait_split_patch()
    if trace:
        _install_ntff_shim()
    from concourse.bass_utils import run_bass_kernel_spmd

    wmap = _prep_weights(inputs)
    x = np.asarray(inputs["x"], F32)
    in_maps = []
    for cid in range(NCORES):
        m = dict(wmap)
        m.update(_prep_x(x[cid * BC:(cid + 1) * BC]))
        in_maps.append(m)

    nc = _build_program(steps=steps, hist=hist)
    res = run_bass_kernel_spmd(nc, in_maps, list(range(NCORES)), trace=trace)

    l_states = np.empty((steps, B, 4), F32)
    activations = np.empty((steps, B, 2), F32)
    for cid in range(NCORES):
        lo = np.asarray(res.results[cid]["l_out"])    # (128, steps*16)
        ao = np.asarray(res.results[cid]["a_out"])    # (128, steps*8)
        # col = t*16 + k*4 + c ; batch b = c*128 + p
        lo = lo.reshape(128, steps, 4, 4)             # (p, t, k, c)
        l_states[:, cid * BC:(cid + 1) * BC, :] = (
            lo.transpose(1, 3, 0, 2).reshape(steps, BC, 4))
        ao = ao.reshape(128, steps, 4, 2)             # (p, t, c, k)
        activations[:, cid * BC:(cid + 1) * BC, :] = (
            ao.transpose(1, 2, 0, 3).reshape(steps, BC, 2))
    return (l_states, activations), res


def kernel(**inputs):
    (l_states, activations), _ = _run(inputs)
    return l_states, activations
